# revision 1
# baseline (speedup 1.0000x reference)
"""GCNConv on 8 TRN2 NeuronCores.

out = rownorm(A + I) @ (x @ W) + b   with A = dense scatter (set semantics)
    = [per dst row r: (sum_{c in dedup(nbr(r))} x[c] + x[r]) / (deg(r)+1)] @ W + b

Strategy (1D node partition, per the sharding hint):
  - host: dedup edges, partition dst rows into 8 contiguous blocks of 2048,
    degree-sort rows inside each core block into 16 tiles of 128 rows,
    build a padded-CSR gather-index array [128, sum(K_t)] per core
    (pad slots point at a zeroed row), plus 1/(deg+1) per row.
  - device (identical program on all 8 cores, different data):
      * cast x f32 -> fp16 into a DRAM scratch (one SWDGE cast DMA)
      * per 2-tile group: one indirect-DMA gather of neighbor rows into
        SBUF [128, K*32] fp16 (one descriptor per edge slot)
      * DVE halving-tree segment sum -> S [128,32] f32
      * PE transpose -> S^T, PE matmul S@W, scalar scale by 1/(deg+1),
        DVE bias add, DMA out
  - host: inverse-permute the 8x2048 row blocks into the full output.
"""

import numpy as np
from contextlib import ExitStack

N = 16384
E = 524288
D = 32
P = 128
NCORES = 8
RPC = N // NCORES          # rows per core = 2048
NTILES = RPC // P          # 16 tiles of 128 rows per core
GROUP = 1                  # tiles per gather instruction
ZROW = N                   # index of the zeroed pad row in the fp16 scratch

_CACHE = {}
_PREP_CACHE = {}
LAST_RESULTS = None        # BassKernelResults of the last run (for test.py)
_TRACE = False             # test.py can flip this for a profiled run


def _chunks(Ks):
    """Split each tile's K slots into two halves -> 2*NTILES gather chunks.
    Returns [(tile, slot_lo, slot_hi)]."""
    out = []
    for t, K in enumerate(Ks):
        h = K // 2
        out.append((t, 0, h))
        out.append((t, h, K))
    return out


def _preprocess(edge_index):
    """Dedup edges, build per-core degree-sorted padded-CSR gather schedule."""
    ei = np.asarray(edge_index)
    key = ei.tobytes()
    if key in _PREP_CACHE:
        return _PREP_CACHE[key]

    dst = ei[0].astype(np.int64)
    src = ei[1].astype(np.int64)
    keys = np.unique(dst * N + src)          # set semantics
    d = (keys // N).astype(np.int64)
    s = (keys % N).astype(np.int32)
    rowptr = np.searchsorted(d, np.arange(N + 1)).astype(np.int64)
    deg = np.diff(rowptr)                    # distinct out-neighbors per row
    slots = (deg + 1).astype(np.int64)       # + self loop
    inv = (1.0 / slots).astype(np.float32)

    # per-core degree-descending row order
    perms = []
    for c in range(NCORES):
        rows = np.arange(c * RPC, (c + 1) * RPC)
        order = np.argsort(-slots[rows], kind="stable")
        perms.append(rows[order])

    # shared (SPMD) per-tile pad width: max slots across cores in that tile
    Ks = []
    for t in range(NTILES):
        m = max(int(slots[perms[c][t * P]]) for c in range(NCORES))
        Ks.append(max(m, 2))
    Ks = tuple(Ks)
    offs = np.concatenate([[0], np.cumsum(Ks)]).astype(np.int64)
    SUMK = int(offs[-1])

    ngroups = NTILES // GROUP
    idx_arrs, inv_arrs = [], []
    for c in range(NCORES):
        plain = np.full((P, SUMK), ZROW, np.int16)
        invt = np.zeros((P, NTILES), np.float32)
        pc = perms[c]
        for t in range(NTILES):
            o = int(offs[t])
            for p in range(P):
                r = int(pc[t * P + p])
                a, b = rowptr[r], rowptr[r + 1]
                k = int(b - a)
                plain[p, o:o + k] = s[a:b]
                plain[p, o + k] = r          # self loop slot
                invt[p, t] = inv[r]
        # dma_gather index format: per gather chunk, gathered position
        # i = j*128 + p reads wrapped[i%16, i//16]; wrapped block for a chunk
        # at slot columns [a, b) of tile t occupies idxw columns
        # [8*(off_t+a), 8*(off_t+b)); replicated to all 128 partitions
        # (one copy per GPSIMD core's partition group).
        idxw = np.empty((16, 8 * SUMK), np.int16)
        for (t, a, b) in _chunks(Ks):
            o = int(offs[t]) + a
            block = plain[:, o:o + (b - a)]       # [128, Kc]
            flat = block.T.reshape(-1)            # flat[j*128+p] = block[p, j]
            idxw[:, 8 * o:8 * (o + (b - a))] = flat.reshape(-1, 16).T
        idx_arrs.append(np.ascontiguousarray(np.tile(idxw, (8, 1))))
        inv_arrs.append(invt)

    prep = {
        "Ks": Ks,
        "offs": offs,
        "SUMK": SUMK,
        "idx": idx_arrs,
        "inv": inv_arrs,
        "perm": perms,
    }
    _PREP_CACHE[key] = prep
    return prep


def _emit_dma_gather(nc, out_ap, in_ap, idxs_ap, num_idxs, elem_size, elem_step,
                     queue_num=0):
    """bass.dma_gather minus its elem_size_bytes%256 assert (that restriction
    is transpose-only; the real ISA constraint is the source stride, which is
    encoded in 256B units and satisfied by the 256B-pitch scratch)."""
    from concourse import mybir
    from concourse._compat import exact_div

    eng = nc.gpsimd
    assert in_ap.ap[0][0] == elem_step
    stride_bytes = elem_step * mybir.dt.size(in_ap.dtype)
    stride_bytes_256 = exact_div(stride_bytes, 256)
    _in_ap = eng.lower_ap_dma(in_ap, for_custom_bir_dma=True)
    _idxs_ap = eng.lower_ap(idxs_ap)
    _out_ap = eng.lower_ap(out_ap)
    return eng.add_instruction(
        mybir.InstDMAGatherAnt(
            name=nc.get_next_instruction_name(),
            ins=[*_in_ap, _idxs_ap, eng.lower_val_access(eng.to_reg(num_idxs))],
            outs=[_out_ap],
            transpose=False,
            num_idxs=num_idxs,
            elem_size=elem_size,
            stride_bytes_256=stride_bytes_256,
            gen_mode=0,
            single_packet=False,
            queue_num=queue_num,
            sbuf_tokens_per_rank=0,
            sbuf_free_dim_per_rank=0,
            sbuf_free_dim_pad_per_rank=0,
            sbuf_byte_offset=0,
        )
    )


PITCH = 128  # fp16 elems per scratch row = 256B (ISA stride granularity)


def _build(Ks, SUMK):
    """Build + compile the (identical-across-cores) Bass program."""
    from concourse import bass, bacc, mybir, tile
    from concourse.masks import make_identity

    ck = (Ks, SUMK)
    if ck in _CACHE:
        return _CACHE[ck]

    f32 = mybir.dt.float32
    f16 = mybir.dt.float16
    i16 = mybir.dt.int16

    nc = bacc.Bacc(
        "TRN2",
        target_bir_lowering=False,
        debug=False,
        enable_asserts=False,
        num_devices=NCORES,
        num_swdge_queues=4,
    )

    x32 = nc.dram_tensor("x32", [N, D], f32, kind="ExternalInput").ap()
    idx_d = nc.dram_tensor("idx", [P, 8 * SUMK], i16, kind="ExternalInput").ap()
    inv_d = nc.dram_tensor("inv", [P, NTILES], f32, kind="ExternalInput").ap()
    w_d = nc.dram_tensor("w", [D, D], f32, kind="ExternalInput").ap()
    bias_d = nc.dram_tensor("biasrep", [P, D], f32, kind="ExternalInput").ap()
    out_d = nc.dram_tensor("out", [RPC, D], f32, kind="ExternalOutput").ap()
    x16_d = nc.dram_tensor("x16s", [N + 1, PITCH], f16, kind="Internal").ap()

    offs = np.concatenate([[0], np.cumsum(Ks)]).astype(np.int64)

    with tile.TileContext(nc) as tc, ExitStack() as ctx:
        const = ctx.enter_context(tc.tile_pool(name="const", bufs=1))
        gp = ctx.enter_context(tc.tile_pool(name="gp", bufs=6))
        sp = ctx.enter_context(tc.tile_pool(name="sp", bufs=3))
        tp = ctx.enter_context(tc.tile_pool(name="tp", bufs=3))
        op_ = ctx.enter_context(tc.tile_pool(name="op", bufs=3))
        ppt = ctx.enter_context(tc.tile_pool(name="ppt", bufs=2, space="PSUM"))
        ppm = ctx.enter_context(tc.tile_pool(name="ppm", bufs=2, space="PSUM"))

        # constants
        w_sb = const.tile([D, D], f32)
        nc.sync.dma_start(out=w_sb[:], in_=w_d[:])
        bias_sb = const.tile([P, D], f32)
        nc.sync.dma_start(out=bias_sb[:], in_=bias_d[:])
        inv_sb = const.tile([P, NTILES], f32)
        nc.sync.dma_start(out=inv_sb[:], in_=inv_d[:])
        idx_sb = const.tile([P, 8 * SUMK], i16)
        nc.sync.dma_start(out=idx_sb[:], in_=idx_d[:])
        ident = const.tile([P, P], f32)
        make_identity(nc, ident[:])

        # zero the pad row of the fp16 scratch
        zrow = const.tile([1, D], f16)
        nc.vector.memset(zrow[:], 0.0)
        nc.sync.dma_start(out=x16_d[ZROW:ZROW + 1, 0:D], in_=zrow[:])

        # cast+pad x f32 -> fp16 into 256B-pitch scratch rows via HWDGE+DVE
        # (keeps the SWDGE queues free for the gathers)
        NSPLIT = 4
        A_ = (N // P) // NSPLIT          # rows per partition per chunk
        x32v = x32.rearrange("(p a) d -> p a d", p=P)
        x16v = x16_d[0:N, 0:D].rearrange("(p a) d -> p a d", p=P)
        for i in range(NSPLIT):
            xt = gp.tile([P, A_ * D], f32, tag="xt", bufs=2)
            nc.sync.dma_start(out=xt[:], in_=x32v[:, i * A_:(i + 1) * A_, :])
            xc = gp.tile([P, A_ * D], f16, tag="xc", bufs=2)
            nc.vector.tensor_copy(out=xc[:], in_=xt[:])
            nc.sync.dma_start(
                out=x16v[:, i * A_:(i + 1) * A_, :],
                in_=xc[:].rearrange("p (a d) -> p a d", d=D),
            )

        # Balance the 4 SWDGE queues by descriptor count (equal-cardinality
        # 4-partition, LPT + swap improvement). Tile's DMASW sem-lane
        # assignment requires queue == emission position % 4, so emit the
        # queues' tiles interleaved round-robin.
        NQ = 4
        per_q = NTILES // NQ
        qlists = [[] for _ in range(NQ)]
        for t in sorted(range(NTILES), key=lambda t: -Ks[t]):
            cands = [q for q in range(NQ) if len(qlists[q]) < per_q]
            q = min(cands, key=lambda i: sum(Ks[x] for x in qlists[i]))
            qlists[q].append(t)
        improved = True
        while improved:
            improved = False
            loads = [sum(Ks[x] for x in l) for l in qlists]
            hi = max(range(NQ), key=lambda q: loads[q])
            lo = min(range(NQ), key=lambda q: loads[q])
            for a in qlists[hi]:
                for b in qlists[lo]:
                    delta = Ks[a] - Ks[b]
                    if 0 < delta < loads[hi] - loads[lo]:
                        qlists[hi].remove(a)
                        qlists[lo].remove(b)
                        qlists[hi].append(b)
                        qlists[lo].append(a)
                        improved = True
                        break
                if improved:
                    break
        # Emit gathers: each tile is split into two chunk-gathers (small
        # enough that the Q7 never stalls on ring space), both chunks of a
        # round's tiles back-to-back, queues strictly round-robin so the
        # DMASW sem-lane/queue pairing stays consistent.
        def emit_chunk(t, a, b, q):
            o = int(offs[t]) + a
            Kc = b - a
            G = Gt_of[t]
            _emit_dma_gather(
                nc,
                out_ap=G[:, a * D:b * D].rearrange("p (k d) -> p k d", d=D),
                in_ap=x16_d[:, 0:D],
                idxs_ap=idx_sb[:, 8 * o:8 * (o + Kc)],
                num_idxs=P * Kc,
                elem_size=D,
                elem_step=PITCH,
                queue_num=q,
            )

        Gt_of = {}
        for r in range(per_q):
            rtiles = [qlists[q][r] for q in range(NQ)]
            for t in rtiles:
                Gt_of[t] = gp.tile([P, Ks[t] * D], f16, tag="G",
                                   name=f"G{t}", bufs=8)
            for q, t in enumerate(rtiles):
                emit_chunk(t, 0, Ks[t] // 2, q)
            for q, t in enumerate(rtiles):
                emit_chunk(t, Ks[t] // 2, Ks[t], q)
            for t in rtiles:
                K = Ks[t]
                Gt = Gt_of[t][:, 0:K * D]

                # halving-tree segment sum over the K slot blocks (fp16),
                # final level lands in f32
                S = sp.tile([P, D], f32, tag="S")
                cur = K
                while cur > 2:
                    if cur % 2 == 1:
                        nc.vector.tensor_add(
                            out=Gt[:, 0:D],
                            in0=Gt[:, 0:D],
                            in1=Gt[:, (cur - 1) * D:cur * D],
                        )
                        cur -= 1
                    else:
                        m = cur // 2
                        nc.vector.tensor_add(
                            out=Gt[:, 0:m * D],
                            in0=Gt[:, 0:m * D],
                            in1=Gt[:, m * D:2 * m * D],
                        )
                        cur = m
                nc.vector.tensor_add(out=S[:], in0=Gt[:, 0:D], in1=Gt[:, D:2 * D])

                # S^T via PE, then (S @ W) via PE
                pT = ppt.tile([D, P], f32, tag="pT")
                nc.tensor.transpose(out=pT[:], in_=S[:], identity=ident[:])
                ST = tp.tile([D, P], f32, tag="ST")
                nc.scalar.copy(out=ST[:], in_=pT[:])
                pO = ppm.tile([P, D], f32, tag="pO")
                nc.tensor.matmul(
                    out=pO[:], lhsT=ST[:], rhs=w_sb[:], start=True, stop=True
                )

                # scale by 1/(deg+1) (per-partition), + bias, store
                O = op_.tile([P, D], f32, tag="O")
                nc.scalar.activation(
                    out=O[:],
                    in_=pO[:],
                    func=mybir.ActivationFunctionType.Copy,
                    scale=inv_sb[:, t:t + 1],
                )
                nc.vector.tensor_add(out=O[:], in0=O[:], in1=bias_sb[:])
                nc.sync.dma_start(out=out_d[t * P:(t + 1) * P, :], in_=O[:])

    nc.compile()
    _CACHE[ck] = nc
    return nc


def kernel(**inputs):
    global LAST_RESULTS
    from concourse import bass_utils

    x = np.ascontiguousarray(np.asarray(inputs["x"], dtype=np.float32))
    edge_index = np.asarray(inputs["edge_index"])
    weight = np.ascontiguousarray(np.asarray(inputs["weight"], dtype=np.float32))
    bias = np.asarray(inputs["bias"], dtype=np.float32)

    prep = _preprocess(edge_index)
    nc = _build(prep["Ks"], prep["SUMK"])

    bias_rep = np.ascontiguousarray(np.broadcast_to(bias[None, :], (P, D)))
    in_maps = [
        {
            "x32": x,
            "idx": prep["idx"][c],
            "inv": prep["inv"][c],
            "w": weight,
            "biasrep": bias_rep,
        }
        for c in range(NCORES)
    ]

    res = bass_utils.run_bass_kernel_spmd(
        nc, in_maps, core_ids=list(range(NCORES)), trace=_TRACE
    )
    LAST_RESULTS = res

    out = np.empty((N, D), dtype=np.float32)
    for c in range(NCORES):
        out[prep["perm"][c]] = res.results[c]["out"]
    return out



# revision 5
# speedup vs baseline: 1.2029x; 1.2029x over previous
"""GCNConv on 8 TRN2 NeuronCores.

out = rownorm(A + I) @ (x @ W) + b   with A = dense scatter (set semantics)
    = [per dst row r: (sum_{c in dedup(nbr(r))} x[c] + x[r]) / (deg(r)+1)] @ W + b

Strategy (1D node partition, per the sharding hint):
  - host: dedup edges, partition dst rows into 8 contiguous blocks of 2048,
    degree-sort rows inside each core block into 16 tiles of 128 rows,
    build a padded-CSR gather-index array [128, sum(K_t)] per core
    (pad slots point at a zeroed row), plus 1/(deg+1) per row.
  - device (identical program on all 8 cores, different data):
      * cast x f32 -> fp16 into a DRAM scratch (one SWDGE cast DMA)
      * per 2-tile group: one indirect-DMA gather of neighbor rows into
        SBUF [128, K*32] fp16 (one descriptor per edge slot)
      * DVE halving-tree segment sum -> S [128,32] f32
      * PE transpose -> S^T, PE matmul S@W, scalar scale by 1/(deg+1),
        DVE bias add, DMA out
  - host: inverse-permute the 8x2048 row blocks into the full output.
"""

import numpy as np
from contextlib import ExitStack

N = 16384
E = 524288
D = 32
P = 128
NCORES = 8
RPC = N // NCORES          # rows per core = 2048
NTILES = RPC // P          # 16 tiles of 128 rows per core
GROUP = 1                  # tiles per gather instruction
ZROW = N                   # index of the zeroed pad row in the fp16 scratch

_CACHE = {}
_PREP_CACHE = {}
LAST_RESULTS = None        # BassKernelResults of the last run (for test.py)
_TRACE = False             # test.py can flip this for a profiled run


def _chunks(Ks):
    """Split each tile's K slots into two halves -> 2*NTILES gather chunks.
    Returns [(tile, slot_lo, slot_hi)]."""
    out = []
    for t, K in enumerate(Ks):
        h = K // 2
        out.append((t, 0, h))
        out.append((t, h, K))
    return out


def _preprocess(edge_index):
    """Dedup edges, build per-core degree-sorted padded-CSR gather schedule."""
    ei = np.asarray(edge_index)
    key = ei.tobytes()
    if key in _PREP_CACHE:
        return _PREP_CACHE[key]

    dst = ei[0].astype(np.int64)
    src = ei[1].astype(np.int64)
    keys = np.unique(dst * N + src)          # set semantics
    d = (keys // N).astype(np.int64)
    s = (keys % N).astype(np.int32)
    rowptr = np.searchsorted(d, np.arange(N + 1)).astype(np.int64)
    deg = np.diff(rowptr)                    # distinct out-neighbors per row
    slots = (deg + 1).astype(np.int64)       # + self loop
    inv = (1.0 / slots).astype(np.float32)

    # per-core degree-descending row order
    perms = []
    for c in range(NCORES):
        rows = np.arange(c * RPC, (c + 1) * RPC)
        order = np.argsort(-slots[rows], kind="stable")
        perms.append(rows[order])

    # shared (SPMD) per-tile pad width: max slots across cores in that tile
    Ks = []
    for t in range(NTILES):
        m = max(int(slots[perms[c][t * P]]) for c in range(NCORES))
        Ks.append(max(m, 2))
    Ks = tuple(Ks)
    offs = np.concatenate([[0], np.cumsum(Ks)]).astype(np.int64)
    SUMK = int(offs[-1])

    ngroups = NTILES // GROUP
    idx_arrs, inv_arrs = [], []
    for c in range(NCORES):
        plain = np.full((P, SUMK), ZROW, np.int16)
        invt = np.zeros((P, NTILES), np.float32)
        pc = perms[c]
        for t in range(NTILES):
            o = int(offs[t])
            for p in range(P):
                r = int(pc[t * P + p])
                a, b = rowptr[r], rowptr[r + 1]
                k = int(b - a)
                plain[p, o:o + k] = s[a:b]
                plain[p, o + k] = r          # self loop slot
                invt[p, t] = inv[r]
        # dma_gather index format: per gather chunk, gathered position
        # i = j*128 + p reads wrapped[i%16, i//16]; wrapped block for a chunk
        # at slot columns [a, b) of tile t occupies idxw columns
        # [8*(off_t+a), 8*(off_t+b)); replicated to all 128 partitions
        # (one copy per GPSIMD core's partition group).
        idxw = np.empty((16, 8 * SUMK), np.int16)
        for (t, a, b) in _chunks(Ks):
            o = int(offs[t]) + a
            block = plain[:, o:o + (b - a)]       # [128, Kc]
            flat = block.T.reshape(-1)            # flat[j*128+p] = block[p, j]
            idxw[:, 8 * o:8 * (o + (b - a))] = flat.reshape(-1, 16).T
        idx_arrs.append(np.ascontiguousarray(np.tile(idxw, (8, 1))))
        inv_arrs.append(invt)

    prep = {
        "Ks": Ks,
        "offs": offs,
        "SUMK": SUMK,
        "idx": idx_arrs,
        "inv": inv_arrs,
        "perm": perms,
    }
    _PREP_CACHE[key] = prep
    return prep


def _emit_dma_gather(nc, out_ap, in_ap, idxs_ap, num_idxs, elem_size, elem_step,
                     queue_num=0):
    """bass.dma_gather minus its elem_size_bytes%256 assert (that restriction
    is transpose-only; the real ISA constraint is the source stride, which is
    encoded in 256B units and satisfied by the 256B-pitch scratch)."""
    from concourse import mybir
    from concourse._compat import exact_div

    eng = nc.gpsimd
    assert in_ap.ap[0][0] == elem_step
    stride_bytes = elem_step * mybir.dt.size(in_ap.dtype)
    stride_bytes_256 = exact_div(stride_bytes, 256)
    _in_ap = eng.lower_ap_dma(in_ap, for_custom_bir_dma=True)
    _idxs_ap = eng.lower_ap(idxs_ap)
    _out_ap = eng.lower_ap(out_ap)
    return eng.add_instruction(
        mybir.InstDMAGatherAnt(
            name=nc.get_next_instruction_name(),
            ins=[*_in_ap, _idxs_ap, eng.lower_val_access(eng.to_reg(num_idxs))],
            outs=[_out_ap],
            transpose=False,
            num_idxs=num_idxs,
            elem_size=elem_size,
            stride_bytes_256=stride_bytes_256,
            gen_mode=0,
            single_packet=False,
            queue_num=queue_num,
            sbuf_tokens_per_rank=0,
            sbuf_free_dim_per_rank=0,
            sbuf_free_dim_pad_per_rank=0,
            sbuf_byte_offset=0,
        )
    )


PITCH = 128  # fp16 elems per scratch row = 256B (ISA stride granularity)


def _build(Ks, SUMK):
    """Build + compile the (identical-across-cores) Bass program."""
    from concourse import bass, bacc, mybir, tile
    from concourse.masks import make_identity

    ck = (Ks, SUMK)
    if ck in _CACHE:
        return _CACHE[ck]

    f32 = mybir.dt.float32
    f16 = mybir.dt.float16
    i16 = mybir.dt.int16

    nc = bacc.Bacc(
        "TRN2",
        target_bir_lowering=False,
        debug=False,
        enable_asserts=False,
        num_devices=NCORES,
        num_swdge_queues=4,
        dynamic_dma_scratch_size=65536,
    )

    x16_d = nc.dram_tensor("x16s", [N + 1, PITCH], f16, kind="ExternalInput").ap()
    idx_d = nc.dram_tensor("idx", [P, 8 * SUMK], i16, kind="ExternalInput").ap()
    inv_d = nc.dram_tensor("inv", [P, NTILES], f32, kind="ExternalInput").ap()
    w_d = nc.dram_tensor("w", [D, D], f32, kind="ExternalInput").ap()
    bias_d = nc.dram_tensor("biasrep", [P, D], f32, kind="ExternalInput").ap()
    out_d = nc.dram_tensor("out", [RPC, D], f32, kind="ExternalOutput").ap()

    offs = np.concatenate([[0], np.cumsum(Ks)]).astype(np.int64)

    with tile.TileContext(nc) as tc, ExitStack() as ctx:
        const = ctx.enter_context(tc.tile_pool(name="const", bufs=1))
        gp = ctx.enter_context(tc.tile_pool(name="gp", bufs=6))
        sp = ctx.enter_context(tc.tile_pool(name="sp", bufs=3))
        tp = ctx.enter_context(tc.tile_pool(name="tp", bufs=3))
        op_ = ctx.enter_context(tc.tile_pool(name="op", bufs=3))
        ppt = ctx.enter_context(tc.tile_pool(name="ppt", bufs=2, space="PSUM"))
        ppm = ctx.enter_context(tc.tile_pool(name="ppm", bufs=2, space="PSUM"))

        # constants
        w_sb = const.tile([D, D], f32)
        nc.sync.dma_start(out=w_sb[:], in_=w_d[:])
        bias_sb = const.tile([P, D], f32)
        nc.sync.dma_start(out=bias_sb[:], in_=bias_d[:])
        inv_sb = const.tile([P, NTILES], f32)
        nc.sync.dma_start(out=inv_sb[:], in_=inv_d[:])
        idx_sb = const.tile([P, 8 * SUMK], i16)
        nc.sync.dma_start(out=idx_sb[:], in_=idx_d[:])
        ident = const.tile([P, P], f32)
        make_identity(nc, ident[:])

        # Balance the 4 SWDGE queues by descriptor count (equal-cardinality
        # 4-partition, LPT + swap improvement). Tile's DMASW sem-lane
        # assignment requires queue == emission position % 4, so emit the
        # queues' tiles interleaved round-robin.
        NQ = 4
        per_q = NTILES // NQ
        qlists = [[] for _ in range(NQ)]
        for t in sorted(range(NTILES), key=lambda t: -Ks[t]):
            cands = [q for q in range(NQ) if len(qlists[q]) < per_q]
            q = min(cands, key=lambda i: sum(Ks[x] for x in qlists[i]))
            qlists[q].append(t)
        improved = True
        while improved:
            improved = False
            loads = [sum(Ks[x] for x in l) for l in qlists]
            hi = max(range(NQ), key=lambda q: loads[q])
            lo = min(range(NQ), key=lambda q: loads[q])
            for a in qlists[hi]:
                for b in qlists[lo]:
                    delta = Ks[a] - Ks[b]
                    if 0 < delta < loads[hi] - loads[lo]:
                        qlists[hi].remove(a)
                        qlists[lo].remove(b)
                        qlists[hi].append(b)
                        qlists[lo].append(a)
                        improved = True
                        break
                if improved:
                    break
        # Emit gathers: each tile is split into two chunk-gathers (small
        # enough that the Q7 never stalls on ring space), both chunks of a
        # round's tiles back-to-back, queues strictly round-robin so the
        # DMASW sem-lane/queue pairing stays consistent.
        def emit_chunk(t, a, b, q):
            o = int(offs[t]) + a
            Kc = b - a
            G = Gt_of[t]
            _emit_dma_gather(
                nc,
                out_ap=G[:, a * D:b * D].rearrange("p (k d) -> p k d", d=D),
                in_ap=x16_d[:, 0:D],
                idxs_ap=idx_sb[:, 8 * o:8 * (o + Kc)],
                num_idxs=P * Kc,
                elem_size=D,
                elem_step=PITCH,
                queue_num=q,
            )

        Gt_of = {}
        for r in range(per_q):
            rtiles = [qlists[q][r] for q in range(NQ)]
            for t in rtiles:
                Gt_of[t] = gp.tile([P, Ks[t] * D], f16, tag="G",
                                   name=f"G{t}", bufs=8)
            for q, t in enumerate(rtiles):
                emit_chunk(t, 0, Ks[t] // 2, q)
            for q, t in enumerate(rtiles):
                emit_chunk(t, Ks[t] // 2, Ks[t], q)
            for t in rtiles:
                K = Ks[t]
                Gt = Gt_of[t][:, 0:K * D]

                # halving-tree segment sum over the K slot blocks (fp16),
                # final level lands in f32
                S = sp.tile([P, D], f32, tag="S")
                cur = K
                while cur > 2:
                    if cur % 2 == 1:
                        nc.vector.tensor_add(
                            out=Gt[:, 0:D],
                            in0=Gt[:, 0:D],
                            in1=Gt[:, (cur - 1) * D:cur * D],
                        )
                        cur -= 1
                    else:
                        m = cur // 2
                        nc.vector.tensor_add(
                            out=Gt[:, 0:m * D],
                            in0=Gt[:, 0:m * D],
                            in1=Gt[:, m * D:2 * m * D],
                        )
                        cur = m
                nc.vector.tensor_add(out=S[:], in0=Gt[:, 0:D], in1=Gt[:, D:2 * D])

                # S^T via PE, then (S @ W) via PE
                pT = ppt.tile([D, P], f32, tag="pT")
                nc.tensor.transpose(out=pT[:], in_=S[:], identity=ident[:])
                ST = tp.tile([D, P], f32, tag="ST")
                nc.scalar.copy(out=ST[:], in_=pT[:])
                pO = ppm.tile([P, D], f32, tag="pO")
                nc.tensor.matmul(
                    out=pO[:], lhsT=ST[:], rhs=w_sb[:], start=True, stop=True
                )

                # scale by 1/(deg+1) (per-partition), + bias, store
                O = op_.tile([P, D], f32, tag="O")
                nc.scalar.activation(
                    out=O[:],
                    in_=pO[:],
                    func=mybir.ActivationFunctionType.Copy,
                    scale=inv_sb[:, t:t + 1],
                )
                nc.vector.tensor_add(out=O[:], in0=O[:], in1=bias_sb[:])
                nc.sync.dma_start(out=out_d[t * P:(t + 1) * P, :], in_=O[:])

    nc.compile()
    _CACHE[ck] = nc
    return nc


def kernel(**inputs):
    global LAST_RESULTS
    from concourse import bass_utils

    x = np.ascontiguousarray(np.asarray(inputs["x"], dtype=np.float32))
    edge_index = np.asarray(inputs["edge_index"])
    weight = np.ascontiguousarray(np.asarray(inputs["weight"], dtype=np.float32))
    bias = np.asarray(inputs["bias"], dtype=np.float32)

    prep = _preprocess(edge_index)
    nc = _build(prep["Ks"], prep["SUMK"])

    # pre-padded fp16 x at 256B row pitch, with a zeroed pad row at index N
    xpad = np.zeros((N + 1, PITCH), dtype=np.float16)
    xpad[:N, :D] = x.astype(np.float16)

    bias_rep = np.ascontiguousarray(np.broadcast_to(bias[None, :], (P, D)))
    in_maps = [
        {
            "x16s": xpad,
            "idx": prep["idx"][c],
            "inv": prep["inv"][c],
            "w": weight,
            "biasrep": bias_rep,
        }
        for c in range(NCORES)
    ]

    res = bass_utils.run_bass_kernel_spmd(
        nc, in_maps, core_ids=list(range(NCORES)), trace=_TRACE
    )
    LAST_RESULTS = res

    out = np.empty((N, D), dtype=np.float32)
    for c in range(NCORES):
        out[prep["perm"][c]] = res.results[c]["out"]
    return out



# revision 19
# speedup vs baseline: 1.6318x; 1.3565x over previous
"""GCNConv on 8 TRN2 NeuronCores.

out = rownorm(A + I) @ (x @ W) + b   with A = dense scatter (set semantics)
    = [per dst row r: (sum_{c in dedup(nbr(r))} x[c] + x[r]) / (deg(r)+1)] @ W + b

Hybrid strategy (1D node partition):
  - host: dedup edges, partition dst rows into 8 contiguous blocks of 2048,
    degree-sort rows inside each core block into 16 tiles of 128 rows.
  - The DENSE_T highest-degree tiles are computed by the PE from a
    host-built fp8 block-dense A^T stream (exact 0/1 weights, fp16 x rhs,
    f32 PSUM accumulation over the 128 source blocks) — no per-edge DMA
    descriptors at all.
  - The remaining low-degree tiles use the indirect-DMA gather path:
    padded-CSR per-slot gather of fp16 x rows (one SWDGE descriptor per
    slot), DVE halving-tree segment sum.
  - Shared tail per tile: PE transpose -> S^T, PE matmul S@W, scalar scale
    by 1/(deg+1), DVE bias add, DMA out.
  - host: inverse-permute the 8x2048 row blocks into the full output.
"""

import numpy as np
from contextlib import ExitStack

N = 16384
E = 524288
D = 32
P = 128
NCORES = 8
RPC = N // NCORES          # rows per core = 2048
NTILES = RPC // P          # 16 tiles of 128 rows per core
NBLK = N // P              # 128 source blocks
DENSE_T = 8                # leading (highest-degree) tiles on the dense path
ZROW = N                   # index of the zeroed pad row in the fp16 scratch
NQ = 4                     # SWDGE queues

_CACHE = {}
_PREP_CACHE = {}
LAST_RESULTS = None        # BassKernelResults of the last run (for test.py)
_TRACE = False             # test.py can flip this for a profiled run

PITCH = 128  # fp16 elems per scratch row = 256B (ISA stride granularity)
FP8_ONE = 0x38  # e4m3 bit pattern for 1.0


def _preprocess(edge_index):
    """Dedup edges; build the dense fp8 A^T stream for the DENSE_T leading
    tiles and a degree-sorted padded-CSR gather schedule for the rest."""
    ei = np.asarray(edge_index)
    key = ei.tobytes()
    if key in _PREP_CACHE:
        return _PREP_CACHE[key]

    dst = ei[0].astype(np.int64)
    src = ei[1].astype(np.int64)
    keys = np.unique(dst * N + src)          # set semantics
    d = (keys // N).astype(np.int64)
    s = (keys % N).astype(np.int32)
    rowptr = np.searchsorted(d, np.arange(N + 1)).astype(np.int64)
    deg = np.diff(rowptr)                    # distinct out-neighbors per row
    slots = (deg + 1).astype(np.int64)       # + self loop
    inv = (1.0 / slots).astype(np.float32)

    # per-core degree-descending row order
    perms = []
    for c in range(NCORES):
        rows = np.arange(c * RPC, (c + 1) * RPC)
        order = np.argsort(-slots[rows], kind="stable")
        perms.append(rows[order])

    g_tiles = list(range(DENSE_T, NTILES))

    # shared (SPMD) per-tile pad width for gathered tiles: max across cores
    Ks = {}
    for t in g_tiles:
        m = max(int(slots[perms[c][t * P]]) for c in range(NCORES))
        Ks[t] = max(m, 2)
    offs = {}
    o = 0
    for t in g_tiles:
        offs[t] = o
        o += Ks[t]
    SUMK = o

    idx_arrs, inv_arrs, a8_arrs = [], [], []
    DCOLS = DENSE_T * P
    for c in range(NCORES):
        # ---- gather-path padded CSR ----
        plain = np.full((P, SUMK), ZROW, np.int16)
        invt = np.zeros((P, NTILES), np.float32)
        pc = perms[c]
        for t in range(NTILES):
            for p in range(P):
                invt[p, t] = inv[pc[t * P + p]]
        for t in g_tiles:
            o = offs[t]
            for p in range(P):
                r = int(pc[t * P + p])
                a, b = rowptr[r], rowptr[r + 1]
                k = int(b - a)
                plain[p, o:o + k] = s[a:b]
                plain[p, o + k] = r          # self loop slot
        # dma_gather index format: per gather chunk, gathered position
        # i = j*128 + p reads wrapped[i%16, i//16]; replicated to all 128
        # partitions (one copy per GPSIMD core's partition group).
        idxw = np.empty((16, 8 * SUMK), np.int16)
        for t in g_tiles:
            K = Ks[t]
            for (a, b) in ((0, K // 2), (K // 2, K)):
                o = offs[t] + a
                block = plain[:, o:o + (b - a)]       # [128, Kc]
                flat = block.T.reshape(-1)            # flat[j*128+p] = block[p, j]
                idxw[:, 8 * o:8 * (o + (b - a))] = flat.reshape(-1, 16).T
        idx_arrs.append(np.ascontiguousarray(np.tile(idxw, (8, 1))))
        inv_arrs.append(invt)

        # ---- dense-path fp8 A^T stream, tile-major, partition-contiguous:
        # a8[s, (t*NBLK + b)*P + j] = weight of edge (b*128+s) -> tile t col j.
        # The reference adds eye on top of the scattered adjacency, so rows
        # with an explicit self-edge get diagonal weight 2.
        dense_rows = pc[:DCOLS].astype(np.int64)      # dst col j holds row
        degs = (rowptr[dense_rows + 1] - rowptr[dense_rows]).astype(np.int64)
        srcs = np.concatenate(
            [np.concatenate([s[rowptr[r]:rowptr[r + 1]] for r in dense_rows]),
             dense_rows]                              # self loops
        )
        jcol = np.arange(DCOLS)
        cols = np.concatenate([np.repeat(jcol, degs), jcol])
        t_of = np.concatenate([np.repeat(jcol // P, degs), jcol // P])
        j_of = np.concatenate([np.repeat(jcol % P, degs), jcol % P])
        cnt = np.zeros((P, DENSE_T * NBLK * P), np.uint8)
        np.add.at(cnt, (srcs % P, (t_of * NBLK + srcs // P) * P + j_of), 1)
        a8 = np.where(cnt == 2, 0x40, np.where(cnt == 1, FP8_ONE, 0)).astype(
            np.uint8
        )
        a8_arrs.append(a8)

    prep = {
        "Ks": tuple(Ks[t] for t in g_tiles),
        "SUMK": SUMK,
        "idx": idx_arrs,
        "inv": inv_arrs,
        "a8": a8_arrs,
        "perm": perms,
    }
    _PREP_CACHE[key] = prep
    return prep


def _emit_dma_gather(nc, out_ap, in_ap, idxs_ap, num_idxs, elem_size, elem_step,
                     queue_num=0):
    """bass.dma_gather minus its elem_size_bytes%256 assert (that restriction
    is transpose-only; the real ISA constraint is the source stride, which is
    encoded in 256B units and satisfied by the 256B-pitch scratch)."""
    from concourse import mybir
    from concourse._compat import exact_div

    eng = nc.gpsimd
    assert in_ap.ap[0][0] == elem_step
    stride_bytes = elem_step * mybir.dt.size(in_ap.dtype)
    stride_bytes_256 = exact_div(stride_bytes, 256)
    _in_ap = eng.lower_ap_dma(in_ap, for_custom_bir_dma=True)
    _idxs_ap = eng.lower_ap(idxs_ap)
    _out_ap = eng.lower_ap(out_ap)
    return eng.add_instruction(
        mybir.InstDMAGatherAnt(
            name=nc.get_next_instruction_name(),
            ins=[*_in_ap, _idxs_ap, eng.lower_val_access(eng.to_reg(num_idxs))],
            outs=[_out_ap],
            transpose=False,
            num_idxs=num_idxs,
            elem_size=elem_size,
            stride_bytes_256=stride_bytes_256,
            gen_mode=0,
            single_packet=False,
            queue_num=queue_num,
            sbuf_tokens_per_rank=0,
            sbuf_free_dim_per_rank=0,
            sbuf_free_dim_pad_per_rank=0,
            sbuf_byte_offset=0,
        )
    )


def _build(Ks, SUMK):
    """Build + compile the (identical-across-cores) Bass program."""
    from concourse import bass, bacc, mybir, tile
    from concourse.masks import make_identity

    ck = (Ks, SUMK)
    if ck in _CACHE:
        return _CACHE[ck]

    f32 = mybir.dt.float32
    f16 = mybir.dt.float16
    f8 = mybir.dt.float8e4
    i16 = mybir.dt.int16

    g_tiles = list(range(DENSE_T, NTILES))
    Kof = {t: Ks[i] for i, t in enumerate(g_tiles)}
    offs = {}
    o = 0
    for t in g_tiles:
        offs[t] = o
        o += Kof[t]
    DCOLS = DENSE_T * P

    nc = bacc.Bacc(
        "TRN2",
        target_bir_lowering=False,
        debug=False,
        enable_asserts=False,
        num_devices=NCORES,
        num_swdge_queues=NQ,
        dynamic_dma_scratch_size=65536,
    )

    x16_d = nc.dram_tensor("x16s", [N + 1, PITCH], f16, kind="ExternalInput").ap()
    idx_d = nc.dram_tensor("idx", [P, 8 * SUMK], i16, kind="ExternalInput").ap()
    inv_d = nc.dram_tensor("inv", [P, NTILES], f32, kind="ExternalInput").ap()
    w_d = nc.dram_tensor("w", [D, D], f32, kind="ExternalInput").ap()
    bias_d = nc.dram_tensor("biasrep", [P, D], f32, kind="ExternalInput").ap()
    a8_d = nc.dram_tensor(
        "a8", [P, DENSE_T * NBLK * P], f8, kind="ExternalInput"
    ).ap()
    # fp8 hi/lo split of x per source block: [x_hi_b | x_lo_b] pairs of D cols
    xblk_d = nc.dram_tensor("xblk", [P, NBLK * 2 * D], f8, kind="ExternalInput").ap()
    out_d = nc.dram_tensor("out", [RPC, D], f32, kind="ExternalOutput").ap()

    with tile.TileContext(nc) as tc, ExitStack() as ctx:
        const = ctx.enter_context(tc.tile_pool(name="const", bufs=1))
        gp = ctx.enter_context(tc.tile_pool(name="gp", bufs=6))
        ap_ = ctx.enter_context(tc.tile_pool(name="ap", bufs=3))
        sp = ctx.enter_context(tc.tile_pool(name="sp", bufs=3))
        tp = ctx.enter_context(tc.tile_pool(name="tp", bufs=3))
        op_ = ctx.enter_context(tc.tile_pool(name="op", bufs=3))
        ppt = ctx.enter_context(tc.tile_pool(name="ppt", bufs=2, space="PSUM"))
        ppm = ctx.enter_context(tc.tile_pool(name="ppm", bufs=2, space="PSUM"))
        ppd = ctx.enter_context(tc.tile_pool(name="ppd", bufs=1, space="PSUM"))

        # constants
        w_sb = const.tile([D, D], f32)
        nc.sync.dma_start(out=w_sb[:], in_=w_d[:])
        bias_sb = const.tile([P, D], f32)
        nc.sync.dma_start(out=bias_sb[:], in_=bias_d[:])
        inv_sb = const.tile([P, NTILES], f32)
        nc.sync.dma_start(out=inv_sb[:], in_=inv_d[:])
        idx_sb = const.tile([P, 8 * SUMK], i16)
        nc.sync.dma_start(out=idx_sb[:], in_=idx_d[:])
        xblk_sb = const.tile([P, NBLK * 2 * D], f8)
        nc.sync.dma_start(out=xblk_sb[:], in_=xblk_d[:])
        ident = const.tile([P, P], f32)
        make_identity(nc, ident[:])

        def tail(t, S_ap):
            # S^T via PE, then (S @ W) via PE, scale by 1/(deg+1), +bias, store
            pT = ppt.tile([D, P], f32, tag="pT")
            nc.tensor.transpose(out=pT[:], in_=S_ap, identity=ident[:])
            ST = tp.tile([D, P], f32, tag="ST")
            nc.scalar.copy(out=ST[:], in_=pT[:])
            pO = ppm.tile([P, D], f32, tag="pO")
            nc.tensor.matmul(
                out=pO[:], lhsT=ST[:], rhs=w_sb[:], start=True, stop=True
            )
            O = op_.tile([P, D], f32, tag="O")
            nc.scalar.activation(
                out=O[:],
                in_=pO[:],
                func=mybir.ActivationFunctionType.Copy,
                scale=inv_sb[:, t:t + 1],
            )
            nc.vector.tensor_add(out=O[:], in0=O[:], in1=bias_sb[:])
            nc.sync.dma_start(out=out_d[t * P:(t + 1) * P, :], in_=O[:])

        # ---------------- dense path: PSUM[t] = sum_b A^T[b,t] @ x[b] -------
        # Tile-major streaming: one PSUM accumulation group open at a time
        # per bank (start=True clears has_written for the WHOLE bank, so
        # interleaved groups in one bank corrupt each other).
        NBC = 32                      # blocks per A^T stream chunk (512KB)
        for t in range(DENSE_T):
            psum_t = ppd.tile([P, 2 * D], f32, tag="pd", bufs=2)
            for i in range(NBLK // NBC):
                a_sb = ap_.tile([P, NBC * P], f8, tag="a8", bufs=4)
                eng = nc.sync if (t * (NBLK // NBC) + i) % 2 else nc.scalar
                eng.dma_start(
                    out=a_sb[:],
                    in_=a8_d[:, (t * NBLK + i * NBC) * P:
                             (t * NBLK + (i + 1) * NBC) * P],
                )
                for k in range(NBC):
                    b = i * NBC + k
                    nc.tensor.matmul(
                        out=psum_t[:],
                        lhsT=a_sb[:, k * P:(k + 1) * P],
                        rhs=xblk_sb[:, b * 2 * D:(b + 1) * 2 * D],
                        start=(b == 0),
                        stop=(b == NBLK - 1),
                    )
            # recombine the fp8 hi/lo halves, then shared tail
            Sd = sp.tile([P, D], f32, tag="S")
            nc.scalar.copy(out=Sd[:], in_=psum_t[:, 0:D])
            nc.vector.tensor_add(out=Sd[:], in0=Sd[:], in1=psum_t[:, D:2 * D])
            tail(t, Sd[:])

        # ---------------- gather path for the low-degree tiles --------------
        # Balance the NQ SWDGE queues by descriptor count (2 tiles per queue,
        # LPT pairing). Tile's DMASW sem-lane assignment keys on emission
        # position; keep queues strictly round-robin in emission order.
        per_q = len(g_tiles) // NQ
        qlists = [[] for _ in range(NQ)]
        for t in sorted(g_tiles, key=lambda t: -Kof[t]):
            cands = [q for q in range(NQ) if len(qlists[q]) < per_q]
            q = min(cands, key=lambda i: sum(Kof[x] for x in qlists[i]))
            qlists[q].append(t)

        def emit_chunk(t, a, b, q):
            o = offs[t] + a
            Kc = b - a
            G = Gt_of[t]
            _emit_dma_gather(
                nc,
                out_ap=G[:, a * D:b * D].rearrange("p (k d) -> p k d", d=D),
                in_ap=x16_d[:, 0:D],
                idxs_ap=idx_sb[:, 8 * o:8 * (o + Kc)],
                num_idxs=P * Kc,
                elem_size=D,
                elem_step=PITCH,
                queue_num=q,
            )

        Gt_of = {}
        for r in range(per_q):
            rtiles = [qlists[q][r] for q in range(NQ)]
            for t in rtiles:
                Gt_of[t] = gp.tile([P, Kof[t] * D], f16, tag="G",
                                   name=f"G{t}", bufs=8)
            for q, t in enumerate(rtiles):
                emit_chunk(t, 0, Kof[t] // 2, q)
            for q, t in enumerate(rtiles):
                emit_chunk(t, Kof[t] // 2, Kof[t], q)
            for t in rtiles:
                K = Kof[t]
                Gt = Gt_of[t][:, 0:K * D]

                # halving-tree segment sum over the K slot blocks (fp16),
                # final level lands in f32
                S = sp.tile([P, D], f32, tag="S")
                cur = K
                while cur > 2:
                    if cur % 2 == 1:
                        nc.vector.tensor_add(
                            out=Gt[:, 0:D],
                            in0=Gt[:, 0:D],
                            in1=Gt[:, (cur - 1) * D:cur * D],
                        )
                        cur -= 1
                    else:
                        m = cur // 2
                        nc.vector.tensor_add(
                            out=Gt[:, 0:m * D],
                            in0=Gt[:, 0:m * D],
                            in1=Gt[:, m * D:2 * m * D],
                        )
                        cur = m
                nc.vector.tensor_add(out=S[:], in0=Gt[:, 0:D], in1=Gt[:, D:2 * D])
                tail(t, S[:])

    nc.compile()
    _CACHE[ck] = nc
    return nc


def kernel(**inputs):
    global LAST_RESULTS
    import ml_dtypes
    from concourse import bass_utils

    x = np.ascontiguousarray(np.asarray(inputs["x"], dtype=np.float32))
    edge_index = np.asarray(inputs["edge_index"])
    weight = np.ascontiguousarray(np.asarray(inputs["weight"], dtype=np.float32))
    bias = np.asarray(inputs["bias"], dtype=np.float32)

    prep = _preprocess(edge_index)
    nc = _build(prep["Ks"], prep["SUMK"])

    x16 = x.astype(np.float16)
    # pre-padded fp16 x at 256B row pitch, with a zeroed pad row at index N
    xpad = np.zeros((N + 1, PITCH), dtype=np.float16)
    xpad[:N, :D] = x16
    # fp8 hi/lo split per source block for the dense rhs:
    # xblk[p, b*2D:(b*2+1)D] = fp8(x[b*128+p]), next D cols = fp8 residual
    f8t = ml_dtypes.float8_e4m3
    xh = x.astype(f8t)
    xl = (x - xh.astype(np.float32)).astype(f8t)
    xblk = np.ascontiguousarray(
        np.concatenate(
            [xh.reshape(NBLK, P, 1, D), xl.reshape(NBLK, P, 1, D)], axis=2
        ).transpose(1, 0, 2, 3).reshape(P, NBLK * 2 * D)
    )

    bias_rep = np.ascontiguousarray(np.broadcast_to(bias[None, :], (P, D)))
    in_maps = [
        {
            "x16s": xpad,
            "idx": prep["idx"][c],
            "inv": prep["inv"][c],
            "w": weight,
            "biasrep": bias_rep,
            "a8": prep["a8"][c].view(ml_dtypes.float8_e4m3),
            "xblk": xblk,
        }
        for c in range(NCORES)
    ]

    res = bass_utils.run_bass_kernel_spmd(
        nc, in_maps, core_ids=list(range(NCORES)), trace=_TRACE
    )
    LAST_RESULTS = res

    out = np.empty((N, D), dtype=np.float32)
    for c in range(NCORES):
        out[prep["perm"][c]] = res.results[c]["out"]
    return out


# revision 25
# speedup vs baseline: 2.2671x; 1.3893x over previous
"""GCNConv on 8 TRN2 NeuronCores.

out = rownorm(A + I) @ (x @ W) + b   with A = dense scatter (set semantics)
    = [per dst row r: (sum_{c in dedup(nbr(r))} x[c] + x[r]) / (deg(r)+1)] @ W + b

Hybrid strategy (1D node partition):
  - host: dedup edges, partition dst rows into 8 contiguous blocks of 2048,
    degree-sort rows inside each core block into 16 tiles of 128 rows.
  - The DENSE_T highest-degree tiles are computed by the PE from a
    host-built fp8 block-dense A^T stream (exact 0/1 weights, fp16 x rhs,
    f32 PSUM accumulation over the 128 source blocks) — no per-edge DMA
    descriptors at all.
  - The remaining low-degree tiles use the indirect-DMA gather path:
    padded-CSR per-slot gather of fp16 x rows (one SWDGE descriptor per
    slot), DVE halving-tree segment sum.
  - Shared tail per tile: PE transpose -> S^T, PE matmul S@W, scalar scale
    by 1/(deg+1), DVE bias add, DMA out.
  - host: inverse-permute the 8x2048 row blocks into the full output.
"""

import numpy as np
from contextlib import ExitStack

N = 16384
E = 524288
D = 32
P = 128
NCORES = 8
RPC = N // NCORES          # rows per core = 2048
NTILES = RPC // P          # 16 tiles of 128 rows per core
NBLK = N // P              # 128 source blocks
DENSE_T = 8                # leading (highest-degree) tiles on the dense path
ZROW = N                   # index of the zeroed pad row in the fp16 scratch
NQ = 4                     # SWDGE queues

_CACHE = {}
_PREP_CACHE = {}
LAST_RESULTS = None        # BassKernelResults of the last run (for test.py)
_TRACE = False             # test.py can flip this for a profiled run

PITCH = 128  # fp16 elems per scratch row = 256B (ISA stride granularity)
FP8_ONE = 0x38  # e4m3 bit pattern for 1.0


def _preprocess(edge_index):
    """Dedup edges; build the dense fp8 A^T stream for the DENSE_T leading
    tiles and a degree-sorted padded-CSR gather schedule for the rest."""
    ei = np.asarray(edge_index)
    key = ei.tobytes()
    if key in _PREP_CACHE:
        return _PREP_CACHE[key]

    dst = ei[0].astype(np.int64)
    src = ei[1].astype(np.int64)
    keys = np.unique(dst * N + src)          # set semantics
    d = (keys // N).astype(np.int64)
    s = (keys % N).astype(np.int32)
    rowptr = np.searchsorted(d, np.arange(N + 1)).astype(np.int64)
    deg = np.diff(rowptr)                    # distinct out-neighbors per row
    slots = (deg + 1).astype(np.int64)       # + self loop
    inv = (1.0 / slots).astype(np.float32)

    # per-core degree-descending row order
    perms = []
    for c in range(NCORES):
        rows = np.arange(c * RPC, (c + 1) * RPC)
        order = np.argsort(-slots[rows], kind="stable")
        perms.append(rows[order])

    g_tiles = list(range(DENSE_T, NTILES))

    # shared (SPMD) per-tile pad width for gathered tiles: max across cores
    Ks = {}
    for t in g_tiles:
        m = max(int(slots[perms[c][t * P]]) for c in range(NCORES))
        Ks[t] = max(m, 2)
    offs = {}
    o = 0
    for t in g_tiles:
        offs[t] = o
        o += Ks[t]
    SUMK = o

    idx_arrs, inv_arrs, a8_arrs = [], [], []
    DCOLS = DENSE_T * P
    for c in range(NCORES):
        # ---- gather-path padded CSR ----
        plain = np.full((P, SUMK), ZROW, np.int16)
        invt = np.zeros((P, NTILES), np.float32)
        pc = perms[c]
        for t in range(NTILES):
            for p in range(P):
                invt[p, t] = inv[pc[t * P + p]]
        for t in g_tiles:
            o = offs[t]
            for p in range(P):
                r = int(pc[t * P + p])
                a, b = rowptr[r], rowptr[r + 1]
                k = int(b - a)
                plain[p, o:o + k] = s[a:b]
                plain[p, o + k] = r          # self loop slot
        # dma_gather index format: per gather chunk, gathered position
        # i = j*128 + p reads wrapped[i%16, i//16]; replicated to all 128
        # partitions (one copy per GPSIMD core's partition group).
        idxw = np.empty((16, 8 * SUMK), np.int16)
        for t in g_tiles:
            K = Ks[t]
            for (a, b) in ((0, K // 2), (K // 2, K)):
                o = offs[t] + a
                block = plain[:, o:o + (b - a)]       # [128, Kc]
                flat = block.T.reshape(-1)            # flat[j*128+p] = block[p, j]
                idxw[:, 8 * o:8 * (o + (b - a))] = flat.reshape(-1, 16).T
        idx_arrs.append(np.ascontiguousarray(np.tile(idxw, (8, 1))))
        inv_arrs.append(invt)

        # ---- dense-path fp8 A^T stream, tile-major, partition-contiguous:
        # a8[s, (t*NBLK + b)*P + j] = weight of edge (b*128+s) -> tile t col j.
        # The reference adds eye on top of the scattered adjacency, so rows
        # with an explicit self-edge get diagonal weight 2.
        dense_rows = pc[:DCOLS].astype(np.int64)      # dst col j holds row
        degs = (rowptr[dense_rows + 1] - rowptr[dense_rows]).astype(np.int64)
        srcs = np.concatenate(
            [np.concatenate([s[rowptr[r]:rowptr[r + 1]] for r in dense_rows]),
             dense_rows]                              # self loops
        )
        jcol = np.arange(DCOLS)
        cols = np.concatenate([np.repeat(jcol, degs), jcol])
        t_of = np.concatenate([np.repeat(jcol // P, degs), jcol // P])
        j_of = np.concatenate([np.repeat(jcol % P, degs), jcol % P])
        cnt = np.zeros((P, DENSE_T * NBLK * P), np.uint8)
        np.add.at(cnt, (srcs % P, (t_of * NBLK + srcs // P) * P + j_of), 1)
        a8 = np.where(cnt == 2, 0x40, np.where(cnt == 1, FP8_ONE, 0)).astype(
            np.uint8
        )
        a8_arrs.append(a8)

    prep = {
        "Ks": tuple(Ks[t] for t in g_tiles),
        "SUMK": SUMK,
        "idx": idx_arrs,
        "inv": inv_arrs,
        "a8": a8_arrs,
        "perm": perms,
    }
    _PREP_CACHE[key] = prep
    return prep


def _emit_dma_gather(nc, out_ap, in_ap, idxs_ap, num_idxs, elem_size, elem_step,
                     queue_num=0):
    """bass.dma_gather minus its elem_size_bytes%256 assert (that restriction
    is transpose-only; the real ISA constraint is the source stride, which is
    encoded in 256B units and satisfied by the 256B-pitch scratch)."""
    from concourse import mybir
    from concourse._compat import exact_div

    eng = nc.gpsimd
    assert in_ap.ap[0][0] == elem_step
    stride_bytes = elem_step * mybir.dt.size(in_ap.dtype)
    stride_bytes_256 = exact_div(stride_bytes, 256)
    _in_ap = eng.lower_ap_dma(in_ap, for_custom_bir_dma=True)
    _idxs_ap = eng.lower_ap(idxs_ap)
    _out_ap = eng.lower_ap(out_ap)
    return eng.add_instruction(
        mybir.InstDMAGatherAnt(
            name=nc.get_next_instruction_name(),
            ins=[*_in_ap, _idxs_ap, eng.lower_val_access(eng.to_reg(num_idxs))],
            outs=[_out_ap],
            transpose=False,
            num_idxs=num_idxs,
            elem_size=elem_size,
            stride_bytes_256=stride_bytes_256,
            gen_mode=0,
            single_packet=False,
            queue_num=queue_num,
            sbuf_tokens_per_rank=0,
            sbuf_free_dim_per_rank=0,
            sbuf_free_dim_pad_per_rank=0,
            sbuf_byte_offset=0,
        )
    )


def _build(Ks, SUMK):
    """Build + compile the (identical-across-cores) Bass program."""
    from concourse import bass, bacc, mybir, tile
    from concourse.masks import make_identity

    ck = (Ks, SUMK)
    if ck in _CACHE:
        return _CACHE[ck]

    f32 = mybir.dt.float32
    f16 = mybir.dt.float16
    f8 = mybir.dt.float8e4
    i16 = mybir.dt.int16

    g_tiles = list(range(DENSE_T, NTILES))
    Kof = {t: Ks[i] for i, t in enumerate(g_tiles)}
    offs = {}
    o = 0
    for t in g_tiles:
        offs[t] = o
        o += Kof[t]
    DCOLS = DENSE_T * P

    nc = bacc.Bacc(
        "TRN2",
        target_bir_lowering=False,
        debug=False,
        enable_asserts=False,
        num_devices=NCORES,
        num_swdge_queues=NQ,
        dynamic_dma_scratch_size=65536,
    )

    x16_d = nc.dram_tensor("x16s", [N + 1, PITCH], f16, kind="ExternalInput").ap()
    idx_d = nc.dram_tensor("idx", [P, 8 * SUMK], i16, kind="ExternalInput").ap()
    inv_d = nc.dram_tensor("inv", [P, NTILES], f32, kind="ExternalInput").ap()
    w_d = nc.dram_tensor("w2", [2 * D, D], f32, kind="ExternalInput").ap()
    bias_d = nc.dram_tensor("biasrep", [P, D], f32, kind="ExternalInput").ap()
    a8_d = nc.dram_tensor(
        "a8", [P, DENSE_T * NBLK * P], f8, kind="ExternalInput"
    ).ap()
    # fp8 hi/lo split of x per source block: [x_hi_b | x_lo_b] pairs of D cols
    xblk_d = nc.dram_tensor("xblk", [P, NBLK * 2 * D], f8, kind="ExternalInput").ap()
    out_d = nc.dram_tensor("out", [RPC, D], f32, kind="ExternalOutput").ap()

    with tile.TileContext(nc) as tc, ExitStack() as ctx:
        const = ctx.enter_context(tc.tile_pool(name="const", bufs=1))
        gp = ctx.enter_context(tc.tile_pool(name="gp", bufs=6))
        ap_ = ctx.enter_context(tc.tile_pool(name="ap", bufs=3))
        sp = ctx.enter_context(tc.tile_pool(name="sp", bufs=3))
        tp = ctx.enter_context(tc.tile_pool(name="tp", bufs=3))
        op_ = ctx.enter_context(tc.tile_pool(name="op", bufs=3))
        ppt = ctx.enter_context(tc.tile_pool(name="ppt", bufs=2, space="PSUM"))
        ppm = ctx.enter_context(tc.tile_pool(name="ppm", bufs=2, space="PSUM"))
        ppd = ctx.enter_context(tc.tile_pool(name="ppd", bufs=1, space="PSUM"))

        # constants
        w2_sb = const.tile([2 * D, D], f32)
        nc.sync.dma_start(out=w2_sb[:], in_=w_d[:])
        bias_sb = const.tile([1, D], f32)
        nc.sync.dma_start(out=bias_sb[:], in_=bias_d[0:1, :])
        ones1 = const.tile([1, P], f32)
        nc.vector.memset(ones1[:], 1.0)
        inv_sb = const.tile([P, NTILES], f32)
        nc.sync.dma_start(out=inv_sb[:], in_=inv_d[:])
        idx_sb = const.tile([P, 8 * SUMK], i16)
        nc.sync.dma_start(out=idx_sb[:], in_=idx_d[:])
        xblk_sb = const.tile([P, NBLK * 2 * D], f8)
        nc.sync.dma_start(out=xblk_sb[:], in_=xblk_d[:])
        ident = const.tile([P, P], f32)
        make_identity(nc, ident[:])

        def tail(t, psrc_ap, w):
            # Sd = rowscale(psrc, 1/(deg+1)); out = Sd @ W(2) + bias.
            # Scalar + PE only (no DVE): bias is seeded into the output PSUM
            # by a 1-row fp32 matmul, inv scaling rides the PSUM->SBUF copy.
            Sd = sp.tile([P, w], f32, tag="S")
            nc.scalar.activation(
                out=Sd[:],
                in_=psrc_ap,
                func=mybir.ActivationFunctionType.Copy,
                scale=inv_sb[:, t:t + 1],
            )
            pT = ppt.tile([w, P], f32, tag="pT")
            nc.tensor.transpose(out=pT[:], in_=Sd[:], identity=ident[:])
            ST = tp.tile([w, P], f32, tag="ST")
            nc.scalar.copy(out=ST[:], in_=pT[:])
            pO = ppm.tile([P, D], f32, tag="pO")
            nc.tensor.matmul(
                out=pO[:], lhsT=ones1[:], rhs=bias_sb[:], start=True, stop=False
            )
            nc.tensor.matmul(
                out=pO[:], lhsT=ST[:], rhs=w2_sb[0:w, :], start=False, stop=True
            )
            O = op_.tile([P, D], f32, tag="O")
            nc.scalar.copy(out=O[:], in_=pO[:])
            nc.sync.dma_start(out=out_d[t * P:(t + 1) * P, :], in_=O[:])

        # ---------------- dense path: PSUM[t] = sum_b A^T[b,t] @ x[b] -------
        # Tile-major streaming: one PSUM accumulation group open at a time
        # per bank (start=True clears has_written for the WHOLE bank, so
        # interleaved groups in one bank corrupt each other).
        NBC = 32                      # blocks per A^T stream chunk (512KB)
        for t in range(DENSE_T):
            psum_t = ppd.tile([P, 2 * D], f32, tag="pd", bufs=2)
            for i in range(NBLK // NBC):
                a_sb = ap_.tile([P, NBC * P], f8, tag="a8", bufs=4)
                eng = nc.sync if (t * (NBLK // NBC) + i) % 2 else nc.scalar
                eng.dma_start(
                    out=a_sb[:],
                    in_=a8_d[:, (t * NBLK + i * NBC) * P:
                             (t * NBLK + (i + 1) * NBC) * P],
                )
                for k in range(NBC):
                    b = i * NBC + k
                    nc.tensor.matmul(
                        out=psum_t[:],
                        lhsT=a_sb[:, k * P:(k + 1) * P],
                        rhs=xblk_sb[:, b * 2 * D:(b + 1) * 2 * D],
                        start=(b == 0),
                        stop=(b == NBLK - 1),
                    )
            # shared tail; the fp8 hi/lo halves recombine inside the W matmul
            # (lhsT = [S_hi^T; S_lo^T], rhs = [W; W])
            tail(t, psum_t[:], 2 * D)

        # ---------------- gather path for the low-degree tiles --------------
        # Balance the NQ SWDGE queues by descriptor count (2 tiles per queue,
        # LPT pairing). Tile's DMASW sem-lane assignment keys on emission
        # position; keep queues strictly round-robin in emission order.
        per_q = len(g_tiles) // NQ
        qlists = [[] for _ in range(NQ)]
        for t in sorted(g_tiles, key=lambda t: -Kof[t]):
            cands = [q for q in range(NQ) if len(qlists[q]) < per_q]
            q = min(cands, key=lambda i: sum(Kof[x] for x in qlists[i]))
            qlists[q].append(t)

        def emit_chunk(t, a, b, q):
            o = offs[t] + a
            Kc = b - a
            G = Gt_of[t]
            _emit_dma_gather(
                nc,
                out_ap=G[:, a * D:b * D].rearrange("p (k d) -> p k d", d=D),
                in_ap=x16_d[:, 0:D],
                idxs_ap=idx_sb[:, 8 * o:8 * (o + Kc)],
                num_idxs=P * Kc,
                elem_size=D,
                elem_step=PITCH,
                queue_num=q,
            )

        Gt_of = {}
        for r in range(per_q):
            rtiles = [qlists[q][r] for q in range(NQ)]
            for t in rtiles:
                Gt_of[t] = gp.tile([P, Kof[t] * D], f16, tag="G",
                                   name=f"G{t}", bufs=8)
            for q, t in enumerate(rtiles):
                emit_chunk(t, 0, Kof[t] // 2, q)
            for q, t in enumerate(rtiles):
                emit_chunk(t, Kof[t] // 2, Kof[t], q)
            for t in rtiles:
                K = Kof[t]
                Gt = Gt_of[t][:, 0:K * D]

                # halving-tree segment sum over the K slot blocks (fp16),
                # final level lands in f32
                S = sp.tile([P, D], f32, tag="S")
                cur = K
                while cur > 2:
                    if cur % 2 == 1:
                        nc.vector.tensor_add(
                            out=Gt[:, 0:D],
                            in0=Gt[:, 0:D],
                            in1=Gt[:, (cur - 1) * D:cur * D],
                        )
                        cur -= 1
                    else:
                        m = cur // 2
                        nc.vector.tensor_add(
                            out=Gt[:, 0:m * D],
                            in0=Gt[:, 0:m * D],
                            in1=Gt[:, m * D:2 * m * D],
                        )
                        cur = m
                nc.vector.tensor_add(out=S[:], in0=Gt[:, 0:D], in1=Gt[:, D:2 * D])
                # Push the gather tails to the very end of every engine's
                # schedule: the scheduler's cost model underestimates the
                # gather DMAs, and an early-queued tail op blocks the engine
                # behind a long semaphore wait (head-of-line).
                with tc.tile_wait_until(1.0):
                    tail(t, S[:], D)

    nc.compile()
    _CACHE[ck] = nc
    return nc


def kernel(**inputs):
    global LAST_RESULTS
    import ml_dtypes
    from concourse import bass_utils

    x = np.ascontiguousarray(np.asarray(inputs["x"], dtype=np.float32))
    edge_index = np.asarray(inputs["edge_index"])
    weight = np.ascontiguousarray(np.asarray(inputs["weight"], dtype=np.float32))
    bias = np.asarray(inputs["bias"], dtype=np.float32)

    prep = _preprocess(edge_index)
    nc = _build(prep["Ks"], prep["SUMK"])

    x16 = x.astype(np.float16)
    # pre-padded fp16 x at 256B row pitch, with a zeroed pad row at index N
    xpad = np.zeros((N + 1, PITCH), dtype=np.float16)
    xpad[:N, :D] = x16
    # fp8 hi/lo split per source block for the dense rhs:
    # xblk[p, b*2D:(b*2+1)D] = fp8(x[b*128+p]), next D cols = fp8 residual
    f8t = ml_dtypes.float8_e4m3
    xh = x.astype(f8t)
    xl = (x - xh.astype(np.float32)).astype(f8t)
    xblk = np.ascontiguousarray(
        np.concatenate(
            [xh.reshape(NBLK, P, 1, D), xl.reshape(NBLK, P, 1, D)], axis=2
        ).transpose(1, 0, 2, 3).reshape(P, NBLK * 2 * D)
    )

    bias_rep = np.ascontiguousarray(np.broadcast_to(bias[None, :], (P, D)))
    in_maps = [
        {
            "x16s": xpad,
            "idx": prep["idx"][c],
            "inv": prep["inv"][c],
            "w2": np.ascontiguousarray(np.vstack([weight, weight])),
            "biasrep": bias_rep,
            "a8": prep["a8"][c].view(ml_dtypes.float8_e4m3),
            "xblk": xblk,
        }
        for c in range(NCORES)
    ]

    res = bass_utils.run_bass_kernel_spmd(
        nc, in_maps, core_ids=list(range(NCORES)), trace=_TRACE
    )
    LAST_RESULTS = res

    out = np.empty((N, D), dtype=np.float32)
    for c in range(NCORES):
        out[prep["perm"][c]] = res.results[c]["out"]
    return out
